# revision 4
# baseline (speedup 1.0000x reference)
"""LRU (Linear Recurrent Unit) forward pass on 8 Trainium2 NeuronCores.

Reference computation (shapes: x [4096, 4, 1024]):
  u        = einsum("nbd,ed->nbe", x, Wi) + bi           # (n, b, 2d)
  u_r, u_i = gamma * u[..., 0::2], gamma * u[..., 1::2]  # complex input per channel
  h[t]     = lam * h[t-1] + u[t]                         # complex diagonal recurrence
  out      = einsum("nbe,de->nbd", [h_r, h_i], Wo) + bo

Strategy:
  - Shard 4-way over batch x 2-way over the channel dim (8 cores). Each core
    computes a partial output for its 512 channels; the host sums the halves.
  - lam = nu * exp(i*theta) is factored into a real magnitude scan (native
    tensor_tensor_scan: state = nu*state + w per partition lane) and
    unit-modulus rotations e^{+-i*theta*t} applied elementwise with
    host-precomputed cos/sin tables (block-local time).
  - Blocks are fully independent: each 256-token block re-scans a 16-token
    warmup window from the previous block instead of chaining carry state
    (nu <= 0.105, so nu^16 ~ 3e-16 is far below fp32 resolution). The host
    prepends 16 zero tokens to x^T.
  - Projections run on the PE in float32r (1 cycle/row); weights are
    pre-transposed/reordered on the host, with gamma and the even/odd
    deinterleave folded into Wi; biases are applied during PSUM eviction on
    the scalar engine.
"""

import sys

sys.path.insert(0, "/opt/trn_rl_repo")

import numpy as np

import concourse.bass as bass
import concourse.mybir as mybir
from concourse.tile import TileContext
from concourse.vector_clock import ScopedClock

N_SEQ = 4096
BATCH = 4
D = 1024
C = D // 2          # channels per core (d-split by 2)
T = 256             # output tokens per block
P = 16              # warmup tokens re-scanned per block
W = T + P           # scanned tokens per block
NBLK = N_SEQ // T
NCT = C // 128      # channel tiles per core (4)
NET = 2 * C // 128  # e-tiles per core (8): 4 real + 4 imag
NDT = D // 128      # d tiles (8)
NMT = D // 128      # output m tiles (8)

F32 = mybir.dt.float32
F32R = mybir.dt.float32r
AF = mybir.ActivationFunctionType
ALU = mybir.AluOpType

# Pre-rotation channel tiles handled by GPSIMD (rest on the vector engine).
GP_PRE = 2

# ---------------------------------------------------------------------------
# Walrus in this container accepts at most ONE sync wait per instruction.
# Patch the Tile tail drain and post-process the BIR to split excess waits
# onto same-engine nops.
# ---------------------------------------------------------------------------
_WAIT_LIMIT = 1
_uid = [0]


def _patched_drain_and_barrier(self, tick_clock, wait_clock):
    probe = self.nc.sync.nop(nofuse=True)
    wait_clock.add_sem_waits(probe.ins, ScopedClock({None: tick_clock.global_clock}))
    si = probe.ins.sync_info
    waits = list(si.on_wait) if si is not None else []
    if si is not None:
        probe.ins.sync_info = mybir.SyncInfo(
            on_wait=waits[:_WAIT_LIMIT], on_update=list(si.on_update)
        )
    for c in range(_WAIT_LIMIT, len(waits), _WAIT_LIMIT):
        nop = self.nc.sync.nop(nofuse=True)
        nop.ins.sync_info = mybir.SyncInfo(
            on_wait=waits[c : c + _WAIT_LIMIT], on_update=[]
        )
    self.nc.sync.drain()
    self.nc.all_engine_barrier()
    assert self.sems is not None
    popped = self.nc._tile_sem_poison_stack.pop()
    assert popped is self._sem_poison
    self.nc.clear_and_free_semaphores(list(self.sems.allocated().values()))
    self.nc.all_engine_barrier()


TileContext._drain_and_barrier = _patched_drain_and_barrier


def _split_sync_waits(nc):
    for func in nc.m.functions:
        for bb in func.blocks:
            out = []
            changed = False
            for ins in bb.instructions:
                si = ins.sync_info
                if si is not None and len(si.on_wait) > _WAIT_LIMIT:
                    waits = list(si.on_wait)
                    for w in waits[:-_WAIT_LIMIT]:
                        _uid[0] += 1
                        nop = mybir.InstNoOp(name=f"wsplit-{_uid[0]}")
                        nop.engine = ins.engine
                        nop.sync_info = mybir.SyncInfo(on_wait=[w], on_update=[])
                        out.append(nop)
                        changed = True
                    ins.sync_info = mybir.SyncInfo(
                        on_wait=waits[-_WAIT_LIMIT:], on_update=list(si.on_update)
                    )
                out.append(ins)
            if changed:
                bb.instructions = out


# ---------------------------------------------------------------------------
# Bass program (identical on all 8 cores; per-core data differs)
# ---------------------------------------------------------------------------
def _build_program():
    nc = bass.Bass()

    xT = nc.dram_tensor("xT", [D, P + N_SEQ], F32R, kind="ExternalInput")
    wiT = nc.dram_tensor("wiT", [D, 2 * C], F32R, kind="ExternalInput")
    woT = nc.dram_tensor("woT", [2 * C, D], F32R, kind="ExternalInput")
    ubias = nc.dram_tensor("ubias", [128, NET], F32, kind="ExternalInput")
    obias = nc.dram_tensor("obias", [128, NMT], F32, kind="ExternalInput")
    # col ct*W + m holds cos/sin(theta_c * (m - P)) for channel ct*128 + p
    cosT = nc.dram_tensor("cosT", [128, NCT * W], F32, kind="ExternalInput")
    sinT = nc.dram_tensor("sinT", [128, NCT * W], F32, kind="ExternalInput")
    nuT = nc.dram_tensor("nuT", [128, NCT * W], F32, kind="ExternalInput")
    outT = nc.dram_tensor("outT", [D, N_SEQ], F32, kind="ExternalOutput")

    with TileContext(nc) as tc:
        with (
            tc.tile_pool(name="const", bufs=1) as cpool,
            tc.tile_pool(name="xt", bufs=2) as xt_pool,
            tc.tile_pool(name="u", bufs=2) as u_pool,
            tc.tile_pool(name="s", bufs=2) as s_pool,
            tc.tile_pool(name="o", bufs=2) as o_pool,
            tc.tile_pool(name="tmp", bufs=3) as tmp_pool,
            tc.tile_pool(name="pu", bufs=3, space="PSUM") as pu_pool,
            tc.tile_pool(name="po", bufs=3, space="PSUM") as po_pool,
        ):
            # resident constants
            wi_sb = []
            for dk in range(NDT):
                t = cpool.tile([128, 2 * C], F32R, tag=f"wi{dk}")
                nc.sync.dma_start(out=t[:], in_=wiT[dk * 128 : (dk + 1) * 128, :])
                wi_sb.append(t)
            wo_sb = []
            for et in range(NET):
                t = cpool.tile([128, D], F32R, tag=f"wo{et}")
                nc.sync.dma_start(out=t[:], in_=woT[et * 128 : (et + 1) * 128, :])
                wo_sb.append(t)
            cos_sb = cpool.tile([128, NCT * W], F32, tag="cos")
            nc.sync.dma_start(out=cos_sb[:], in_=cosT[:])
            sin_sb = cpool.tile([128, NCT * W], F32, tag="sin")
            nc.sync.dma_start(out=sin_sb[:], in_=sinT[:])
            nu_sb = cpool.tile([128, NCT * W], F32, tag="nu")
            nc.sync.dma_start(out=nu_sb[:], in_=nuT[:])
            ub_sb = cpool.tile([128, NET], F32, tag="ubias")
            nc.sync.dma_start(out=ub_sb[:], in_=ubias[:])
            ob_sb = cpool.tile([128, NMT], F32, tag="obias")
            nc.sync.dma_start(out=ob_sb[:], in_=obias[:])

            # strided [128, NCT, T] views of the tables (post-rotation cols)
            cos_post = cos_sb[:].rearrange("p (a b) -> p a b", b=W)[:, :, P:W]
            sin_post = sin_sb[:].rearrange("p (a b) -> p a b", b=W)[:, :, P:W]

            for blk in range(NBLK):
                t0 = blk * T  # column offset into padded xT (= global t - P)
                # ---- load x^T tiles for this block
                xt = []
                for dk in range(NDT):
                    t = xt_pool.tile([128, W], F32R, tag=f"xt{dk}")
                    nc.sync.dma_start(
                        out=t[:], in_=xT[dk * 128 : (dk + 1) * 128, t0 : t0 + W]
                    )
                    xt.append(t)
                # ---- input projection on PE; bias added during eviction
                ur = u_pool.tile([128, NCT * W], F32, tag="ur")
                ui = u_pool.tile([128, NCT * W], F32, tag="ui")
                for et in range(NET):
                    pt = pu_pool.tile([128, W], F32, tag="pu")
                    for dk in range(NDT):
                        nc.tensor.matmul(
                            pt[:],
                            wi_sb[dk][:, et * 128 : (et + 1) * 128],
                            xt[dk][:],
                            start=(dk == 0),
                            stop=(dk == NDT - 1),
                        )
                    ct = et % NCT
                    dst = (ur if et < NCT else ui)[:, ct * W : (ct + 1) * W]
                    nc.scalar.activation(
                        dst, pt[:], AF.Identity, bias=ub_sb[:, et : et + 1]
                    )
                if blk == 0:
                    # zero the 16 warmup tokens (they hold bias-only values
                    # for the nonexistent tokens before t=0)
                    zv = ur[:].rearrange("p (a b) -> p a b", b=W)[:, :, 0:P]
                    nc.vector.memset(zv, 0.0)
                    zv = ui[:].rearrange("p (a b) -> p a b", b=W)[:, :, 0:P]
                    nc.vector.memset(zv, 0.0)
                # ---- pre-rotation in place: u <- e^{-i theta t} * u
                # split across gpsimd (first GP_PRE channel tiles) and DVE
                spans = [
                    (nc.gpsimd, 0, GP_PRE * W),
                    (nc.vector, GP_PRE * W, NCT * W),
                ]
                for eng, lo, hi in spans:
                    ta = tmp_pool.tile([128, NCT * W], F32, tag="ta")
                    tb = tmp_pool.tile([128, NCT * W], F32, tag="tb")
                    eng.tensor_mul(ta[:, lo:hi], sin_sb[:, lo:hi], ui[:, lo:hi])
                    eng.tensor_mul(tb[:, lo:hi], sin_sb[:, lo:hi], ur[:, lo:hi])
                    eng.tensor_mul(ur[:, lo:hi], cos_sb[:, lo:hi], ur[:, lo:hi])
                    eng.tensor_add(ur[:, lo:hi], ur[:, lo:hi], ta[:, lo:hi])
                    eng.tensor_mul(ui[:, lo:hi], cos_sb[:, lo:hi], ui[:, lo:hi])
                    eng.tensor_sub(ui[:, lo:hi], ui[:, lo:hi], tb[:, lo:hi])
                # ---- magnitude scan per channel tile (DVE, fp32 state)
                sr = s_pool.tile([128, NCT * W], F32, tag="sr")
                si = s_pool.tile([128, NCT * W], F32, tag="si")
                for ct in range(NCT):
                    cols = slice(ct * W, (ct + 1) * W)
                    nc.vector.tensor_tensor_scan(
                        sr[:, cols].bitcast(F32R), nu_sb[:, cols], ur[:, cols], 0.0,
                        ALU.mult, ALU.add,
                    )
                    nc.vector.tensor_tensor_scan(
                        si[:, cols].bitcast(F32R), nu_sb[:, cols], ui[:, cols], 0.0,
                        ALU.mult, ALU.add,
                    )
                # ---- post-rotation in place on the T output cols:
                # h = e^{+i theta t} * s
                srv = sr[:].rearrange("p (a b) -> p a b", b=W)[:, :, P:W]
                siv = si[:].rearrange("p (a b) -> p a b", b=W)[:, :, P:W]
                ta = tmp_pool.tile([128, NCT * W], F32, tag="ta")
                tb = tmp_pool.tile([128, NCT * W], F32, tag="tb")
                tav = ta[:].rearrange("p (a b) -> p a b", b=W)[:, :, P:W]
                tbv = tb[:].rearrange("p (a b) -> p a b", b=W)[:, :, P:W]
                nc.vector.tensor_mul(tav, sin_post, siv)
                nc.vector.tensor_mul(tbv, sin_post, srv)
                nc.vector.tensor_mul(srv.bitcast(F32R), cos_post, srv)
                nc.vector.tensor_sub(srv.bitcast(F32R), srv, tav)
                nc.vector.tensor_mul(siv.bitcast(F32R), cos_post, siv)
                nc.vector.tensor_add(siv.bitcast(F32R), siv, tbv)
                # ---- output projection: out^T[m, t] = Wo^T . h
                for mt in range(NMT):
                    pt = po_pool.tile([128, T], F32, tag="po")
                    for et in range(NET):
                        ct = et % NCT
                        src = (sr if et < NCT else si)[
                            :, ct * W + P : (ct + 1) * W
                        ]
                        nc.tensor.matmul(
                            pt[:],
                            wo_sb[et][:, mt * 128 : (mt + 1) * 128],
                            src.bitcast(F32R),
                            start=(et == 0),
                            stop=(et == NET - 1),
                        )
                    ot = o_pool.tile([128, T], F32, tag=f"o{mt}")
                    nc.scalar.activation(
                        ot[:], pt[:], AF.Identity, bias=ob_sb[:, mt : mt + 1]
                    )
                    nc.sync.dma_start(
                        out=outT[mt * 128 : (mt + 1) * 128, blk * T : (blk + 1) * T],
                        in_=ot[:],
                    )

    _split_sync_waits(nc)
    return nc


_CACHED = None


def _get_program():
    global _CACHED
    if _CACHED is None:
        _CACHED = _build_program()
    return _CACHED


# ---------------------------------------------------------------------------
# Host-side sharding / weight preparation
# ---------------------------------------------------------------------------
def make_in_maps(x, nu_log, theta_log, gamma_log, Wi, bi, Wo, bo):
    nu = np.exp(nu_log.astype(np.float64))          # decay magnitude per channel
    theta = np.exp(theta_log.astype(np.float64))    # rotation angle per channel
    gamma = np.exp(-np.exp(gamma_log.astype(np.float64)))

    tt = np.arange(W, dtype=np.float64) - P         # block-local time
    in_maps = []
    for core in range(8):
        bi_idx = core % 4
        half = core // 4
        cs = np.arange(half * C, (half + 1) * C)    # global channels

        xTc = np.zeros((D, P + N_SEQ), dtype=np.float32)
        xTc[:, P:] = x[:, bi_idx, :].T

        g = gamma[cs].astype(np.float32)
        wiTc = np.empty((D, 2 * C), dtype=np.float32)
        wiTc[:, :C] = (Wi[2 * cs, :] * g[:, None]).T
        wiTc[:, C:] = (Wi[2 * cs + 1, :] * g[:, None]).T
        ub = np.concatenate([g * bi[2 * cs], g * bi[2 * cs + 1]]).astype(np.float32)
        ub2 = np.ascontiguousarray(ub.reshape(NET, 128).T)

        woTc = np.empty((2 * C, D), dtype=np.float32)
        woTc[:C, :] = Wo[:, cs].T
        woTc[C:, :] = Wo[:, D + cs].T
        ob = bo.astype(np.float32) if half == 0 else np.zeros(D, np.float32)
        ob2 = np.ascontiguousarray(ob.reshape(NMT, 128).T)

        # tables: [128, NCT*W], col ct*W + m -> channel ct*128 + p, t = m - P
        th = theta[cs].reshape(NCT, 128)             # [NCT, 128]
        ang = th[:, :, None] * tt[None, None, :]     # [NCT, 128, W]
        cosTc = np.cos(ang).transpose(1, 0, 2).reshape(128, NCT * W)
        sinTc = np.sin(ang).transpose(1, 0, 2).reshape(128, NCT * W)
        nuTc = np.broadcast_to(
            nu[cs].reshape(NCT, 128).transpose(1, 0)[:, :, None], (128, NCT, W)
        ).reshape(128, NCT * W)

        in_maps.append(
            {
                "xT": xTc,
                "wiT": wiTc,
                "woT": woTc,
                "ubias": ub2,
                "obias": ob2,
                "cosT": np.ascontiguousarray(cosTc, dtype=np.float32),
                "sinT": np.ascontiguousarray(sinTc, dtype=np.float32),
                "nuT": np.ascontiguousarray(nuTc, dtype=np.float32),
            }
        )
    return in_maps


def assemble_output(results):
    out = np.empty((N_SEQ, BATCH, D), dtype=np.float32)
    for bi_idx in range(BATCH):
        acc = results[bi_idx]["outT"] + results[4 + bi_idx]["outT"]  # [D, N_SEQ]
        out[:, bi_idx, :] = acc.T
    return out


def kernel(x, nu_log, theta_log, gamma_log, Wi, bi, Wo, bo):
    from concourse.bass_utils import run_bass_kernel_spmd

    nc = _get_program()
    in_maps = make_in_maps(x, nu_log, theta_log, gamma_log, Wi, bi, Wo, bo)
    res = run_bass_kernel_spmd(nc, in_maps, list(range(8)))
    return assemble_output(res.results)
